# revision 9
# baseline (speedup 1.0000x reference)
"""Trainium2 Bass kernel for CnnLSTM (conv1x1 -> 2-layer LSTM -> AR decode).

Strategy: pure data parallel over batch (B=256 -> 32 per core x 8 cores).
Device layout is "feature-major": gates live as [128 partitions = G-chunk,
32 free = batch], hidden state as [128 part = h-dim chunk, 2*32], so the
state tile directly provides matmul rhs slices and no transposes are ever
needed.  Matmul operands are fp16; all accumulation/state math is fp32.

Perf structure (v2):
- Gate biases are folded into the PE accumulation via one [8,128] x [8,256]
  indicator matmul per gate group (removes the DVE bias add + a serial hop
  from every cell).
- Warm loop is software-pipelined with a one-step skew: layer1 of step t-1
  runs concurrently with layer0 of step t (h0 double-buffered), and the two
  cells' ACT/DVE ops are emitted interleaved so the engines' in-order
  queues alternate between the two independent chains.
- AR decode feeds xt directly from h1 via the rank-1 weight matrix
  M = conv_w (x) lin_w (xt = relu(M @ h1 + (lin_b*conv_w + conv_b))),
  eliminating the prediction-row round trip (prow -> p16 -> outer product)
  from the recurrence's critical path.
"""

import numpy as np

import concourse.bacc as bacc
import concourse.bass as bass
import concourse.mybir as mybir
import concourse.tile as tile
from concourse import bass_utils
from concourse.bass import ds

F16 = mybir.dt.float16
F32 = mybir.dt.float32
AF = mybir.ActivationFunctionType
ALU = mybir.AluOpType
ET = mybir.EngineType

# PSUM gate-slot order: i,i,f,f,o,o,g,g  (PyTorch gate order along G is i,f,g,o)
GPERM = [0, 1, 2, 3, 6, 7, 4, 5]
P = 128
B = 32  # batch per core
NCORES = 8
T_FULL = 2048
WARM_BODY = 64  # warmup steps per For_i iteration
AR_BODY = 16  # AR steps per For_i iteration


# ---------------------------------------------------------------- host prep


def _relay_hh(W):
    # W [1024, 256] -> lhsT tiles [128, 2048] fp16; col block (k*8+s)*128+j
    # holds W.T[k*128+p, GPERM[s]*128+j]
    Wt = W.T.reshape(2, 128, 8, 128)
    Wt = Wt[:, :, GPERM, :]
    return np.ascontiguousarray(
        Wt.transpose(1, 0, 2, 3).reshape(128, 2048)
    ).astype(np.float16)


def _bias_lhsT(b):
    # [8, 128] f16: biasT[s, p] = b[GPERM[s]*128 + p]
    return np.ascontiguousarray(b.reshape(8, 128)[GPERM]).astype(np.float16)


def prep_shared(inputs):
    f32 = np.float32
    g = lambda n: np.asarray(inputs[n], f32)
    W_ih0, W_hh0 = g("W_ih0"), g("W_hh0")
    W_ih1, W_hh1 = g("W_ih1"), g("W_hh1")
    b0 = g("b_ih0") + g("b_hh0")
    b1 = g("b_ih1") + g("b_hh1")
    conv_w, conv_b = g("conv_w"), g("conv_b")
    lin_w, lin_b = g("lin_w"), g("lin_b")

    # x-projection weights, slot-major: wih0u[p, s*128+j] = W_ih0.T[p, GPERM[s]*128+j]
    Wt0 = W_ih0.T.reshape(64, 8, 128)[:, GPERM, :]
    wih0u = np.ascontiguousarray(Wt0.reshape(64, 1024)).astype(np.float16)

    cw2 = np.tile(conv_w, 2)
    cb2 = np.tile(conv_b, 2)

    # AR head shortcut: xt_pre = M @ h1 where M[p, j] = cw2[p] * lin_w[j].
    # lhsT chunk k is M[:, k*128:(k+1)*128].T
    Mfull = cw2[:, None] * lin_w[0][None, :]  # [128, 256]
    mwT = np.concatenate(
        [np.ascontiguousarray(Mfull[:, k * 128 : (k + 1) * 128].T) for k in range(2)],
        axis=1,
    ).astype(np.float16)

    # bias indicator rhs: bind[k, s*32+b] = 1.0 if s == k
    bind = np.zeros((8, 256), np.float16)
    for s in range(8):
        bind[s, s * 32 : (s + 1) * 32] = 1.0

    return {
        "whh0": _relay_hh(W_hh0),
        "wih1": _relay_hh(W_ih1),
        "whh1": _relay_hh(W_hh1),
        "wih0u": wih0u,
        "biasT0": _bias_lhsT(b0),
        "biasT1": _bias_lhsT(b1),
        "bind": bind,
        "mwT": mwT,
        # interleaved [cw[c], cb[c]] pairs, replicated across partitions
        "cwcb": np.ascontiguousarray(
            np.broadcast_to(
                np.stack([conv_w, conv_b], axis=1).reshape(1, 128), (128, 128)
            )
        ).astype(f32),
        "linwT": np.ascontiguousarray(lin_w[0].reshape(2, 128).T).astype(np.float16),
        "cb2col": (lin_b[0] * cw2 + cb2).astype(f32)[:, None],
        "linbcol": np.full((32, 1), lin_b[0], f32),
    }


def prep_core_input(input_full, core):
    # inpT[p, r*32+b] = input[32*core+b, 64*r + p%64], duplicated rows 64:128
    x = np.asarray(input_full, np.float32)[32 * core : 32 * core + 32]
    x = x.reshape(32, 32, 64)  # [b, r, k]
    one = x.transpose(2, 1, 0).reshape(64, 1024)  # [k, (r b)]
    return np.ascontiguousarray(np.concatenate([one, one], axis=0))


# ---------------------------------------------------------------- device IR


def build_program(T=T_FULL, NP=512, mode="full"):
    assert T % WARM_BODY == 0 and T <= T_FULL
    assert 2 <= NP <= 512
    nc = bacc.Bacc("TRN2", debug=False, enable_asserts=False, num_devices=NCORES)

    def din(name, shape, dt):
        return nc.dram_tensor(name, list(shape), dt, kind="ExternalInput").ap()

    t = {
        "whh0": din("whh0", (128, 2048), F16),
        "wih1": din("wih1", (128, 2048), F16),
        "whh1": din("whh1", (128, 2048), F16),
        "wih0u": din("wih0u", (64, 1024), F16),
        "inpT": din("inpT", (128, 1024), F32),
        "biasT0": din("biasT0", (8, 128), F16),
        "biasT1": din("biasT1", (8, 128), F16),
        "bind": din("bind", (8, 256), F16),
        "mwT": din("mwT", (128, 256), F16),
        "cwcb": din("cwcb", (128, 128), F32),
        "linwT": din("linwT", (128, 2), F16),
        "cb2col": din("cb2col", (128, 1), F32),
        "linbcol": din("linbcol", (32, 1), F32),
    }
    if mode == "warm":
        out_ap = nc.dram_tensor("out", [128, 128], F32, kind="ExternalOutput").ap()
    else:
        out_ap = nc.dram_tensor("out", [32, NP], F32, kind="ExternalOutput").ap()

    with tile.TileContext(nc) as tc:
        _emit(tc, nc, t, out_ap, T, NP, mode)
    nc.compile()
    return nc


def _emit(tc, nc, t, out_ap, T, NP, mode="full"):
    import contextlib
    from itertools import zip_longest

    with contextlib.ExitStack() as ctx:
        const = ctx.enter_context(tc.tile_pool(name="const", bufs=1))

        def load(name, shape, dt):
            tl = const.tile(list(shape), dt, tag=name)
            nc.sync.dma_start(tl[:], t[name])
            return tl

        whh0 = load("whh0", (128, 2048), F16)
        wih1 = load("wih1", (128, 2048), F16)
        whh1 = load("whh1", (128, 2048), F16)
        wih0u = load("wih0u", (64, 1024), F16)
        inpT = load("inpT", (128, 1024), F32)
        biasT0 = load("biasT0", (8, 128), F16)
        biasT1 = load("biasT1", (8, 128), F16)
        bind = load("bind", (8, 256), F16)
        mwT = load("mwT", (128, 256), F16)
        cwcb = load("cwcb", (128, 128), F32)
        linwT = load("linwT", (128, 2), F16)
        cb2col = load("cb2col", (128, 1), F32)
        linbcol = load("linbcol", (32, 1), F32)

        # persistent state; h0 double-buffered for the warm-loop skew
        h0b = [
            const.tile([128, 64], F16, tag=f"h0_{i}", name=f"h0_{i}")
            for i in range(2)
        ]
        c0 = const.tile([128, 64], F32, tag="c0")
        h1 = const.tile([128, 64], F16, tag="h1")
        c1 = const.tile([128, 64], F32, tag="c1")
        for st in (h0b[0], h0b[1], c0, h1, c1):
            nc.vector.memset(st[:], 0.0)
        preds = const.tile([32, NP], F32, tag="preds")

        gpool = ctx.enter_context(tc.tile_pool(name="gates", bufs=2, space="PSUM"))
        spool = ctx.enter_context(tc.tile_pool(name="sg", bufs=2))
        tpool = ctx.enter_context(tc.tile_pool(name="tmp", bufs=2))
        xpool = ctx.enter_context(tc.tile_pool(name="xt", bufs=2))
        appool = ctx.enter_context(tc.tile_pool(name="arp", bufs=1, space="PSUM"))

        def cell_ops(g, tagp, h_dst, c_st, v):
            """Return the cell's ACT/elementwise ops as closures so two cells
            can be emitted interleaved (each engine's in-order queue then
            alternates between the two independent chains).  `v` picks the
            elementwise engine (nc.vector for layer0, nc.gpsimd for layer1) so
            the two chains don't share a single elementwise queue."""
            sg = spool.tile([128, 192], F32, tag=tagp + "s")
            gt = tpool.tile([128, 64], F32, tag=tagp + "g")
            m1 = tpool.tile([128, 64], F32, tag=tagp + "m1")
            m2 = tpool.tile([128, 64], F32, tag=tagp + "m2")
            tcc = tpool.tile([128, 64], F32, tag=tagp + "t")
            return [
                lambda: nc.scalar.activation(sg[:], g[:, 0:192], AF.Sigmoid),
                lambda: nc.scalar.activation(gt[:], g[:, 192:256], AF.Tanh),
                lambda: v.tensor_mul(m1[:], sg[:, 64:128], c_st[:]),
                lambda: v.tensor_mul(m2[:], sg[:, 0:64], gt[:]),
                lambda: v.tensor_add(c_st[:], m1[:], m2[:]),
                lambda: nc.scalar.activation(tcc[:], c_st[:], AF.Tanh),
                lambda: v.tensor_mul(h_dst[:], sg[:, 128:192], tcc[:]),
            ]

        def hh16(g, w, rhs_tile):
            for s in range(8):
                for k in range(2):
                    nc.tensor.matmul(
                        g[:, s * 32 : (s + 1) * 32],
                        lhsT=w[:, (k * 8 + s) * 128 : (k * 8 + s + 1) * 128],
                        rhs=rhs_tile[:, k * 32 : (k + 1) * 32],
                        start=False,
                        stop=False,
                        skip_group_check=True,
                    )

        def bias_mm(g, biasT):
            nc.tensor.matmul(
                g[:], lhsT=biasT[:], rhs=bind[:], start=True, stop=False,
                skip_group_check=True,
            )

        def l0_mms_x(g, xt):
            for s in range(8):
                nc.tensor.matmul(
                    g[:, s * 32 : (s + 1) * 32],
                    lhsT=wih0u[:, s * 128 : (s + 1) * 128],
                    rhs=xt[0:64, :],
                    start=False,
                    stop=(s == 7),
                    skip_group_check=True,
                )

        def l1_mms_ih(g, h0r, last_stop=True):
            for s in range(8):
                for k in range(2):
                    nc.tensor.matmul(
                        g[:, s * 32 : (s + 1) * 32],
                        lhsT=wih1[:, (k * 8 + s) * 128 : (k * 8 + s + 1) * 128],
                        rhs=h0r[:, k * 32 : (k + 1) * 32],
                        start=False,
                        stop=(last_stop and s == 7 and k == 1),
                        skip_group_check=True,
                    )

        def emit_l0(u, xcur, h0r, h0w):
            g = gpool.tile([128, 256], F32, tag="g0")
            bias_mm(g, biasT0)
            hh16(g, whh0, h0r)
            for s in range(8):
                nc.tensor.matmul(
                    g[:, s * 32 : (s + 1) * 32],
                    lhsT=wih0u[:, s * 128 : (s + 1) * 128],
                    rhs=xcur[0:64, u * 32 : (u + 1) * 32],
                    start=False,
                    stop=(s == 7),
                    skip_group_check=True,
                )
            return cell_ops(g, "l0", h0w, c0, nc.vector)

        def emit_l1(h0r):
            g = gpool.tile([128, 256], F32, tag="g1")
            bias_mm(g, biasT1)
            hh16(g, whh1, h1)
            l1_mms_ih(g, h0r)
            return cell_ops(g, "l1", h1, c1, nc.gpsimd)

        preds_ps = appool.tile([32, NP], F32, tag="predps")

        def pred_col(col):
            for k in range(2):
                nc.tensor.matmul(
                    preds_ps[:, ds(col, 1)],
                    lhsT=h1[:, k * 32 : (k + 1) * 32],
                    rhs=linwT[:, k : k + 1],
                    start=(k == 0),
                    stop=(k == 1),
                    skip_group_check=True,
                )

        hints = (ET.PE, ET.DVE, ET.Activation)

        wtrip = T // WARM_BODY
        with tc.For_i(0, wtrip, 1, hint_engines=hints) as iv:
            # ACT scale/bias operands do not support register offsets on HW,
            # so snapshot this body's two (cw, cb) pairs into static tiles
            # via DMA (register offsets on DMA are fine).
            curs = []
            for half in range(2):
                cur = xpool.tile([128, 2], F32, tag="cwcur")
                nc.sync.dma_start(cur[:], cwcb[:, ds(iv * 4 + half * 2, 2)])
                curs.append(cur)
            # conv+relu for the whole body in two bulk ACT ops (instead of a
            # per-step [128,32] ReLU on the cell chains' shared ACT queue)
            xcur = xpool.tile([128, 2048], F16, tag="xcur")
            for half in range(2):
                nc.scalar.activation(
                    xcur[:, half * 1024 : (half + 1) * 1024],
                    inpT[:],
                    AF.Relu,
                    bias=curs[half][:, 1:2],
                    scale=curs[half][:, 0:1],
                )
            for u in range(WARM_BODY):
                ops0 = emit_l0(u, xcur, h0b[(u - 1) % 2], h0b[u % 2])
                # layer1 for step u-1 (reads h0 written by step u-1)
                ops1 = emit_l1(h0b[(u - 1) % 2]) if u > 0 else None
                for a, b in zip_longest(ops0, ops1 or []):
                    if a is not None:
                        a()
                    if b is not None:
                        b()
            # tail: layer1 of the body's last step
            for op in emit_l1(h0b[(WARM_BODY - 1) % 2]):
                op()

        if mode == "warm":
            dbg = const.tile([128, 128], F32, tag="dbg")
            nc.vector.tensor_copy(dbg[:, 0:64], h1[:])
            nc.vector.tensor_copy(dbg[:, 64:128], c1[:])
            nc.sync.dma_start(out_ap, dbg[:])
            return

        pred_col(0)

        # ----- AR decode: serial chain, but with the M-shortcut head and
        # matmul groups ordered so PE pre-runs whatever is data-ready.
        def ar_step(j, col):
            h0r, h0w = h0b[(j + 1) % 2], h0b[j % 2]
            g0 = gpool.tile([128, 256], F32, tag="g0")
            bias_mm(g0, biasT0)
            hh16(g0, whh0, h0r)  # depends h0(t-1): pre-runs during prev cell
            # xt from h1(t-1) via rank-1 M
            xps = appool.tile([128, 32], F32, tag="xps")
            for k in range(2):
                nc.tensor.matmul(
                    xps[:],
                    lhsT=mwT[:, k * 128 : (k + 1) * 128],
                    rhs=h1[:, k * 32 : (k + 1) * 32],
                    start=(k == 0),
                    stop=(k == 1),
                    skip_group_check=True,
                )
            xt = xpool.tile([128, 32], F16, tag="xt")
            nc.scalar.activation(xt[:], xps[:], AF.Relu, bias=cb2col[:])
            l0_mms_x(g0, xt)
            ops0 = cell_ops(g0, "l0", h0w, c0, nc.vector)
            g1 = gpool.tile([128, 256], F32, tag="g1")
            bias_mm(g1, biasT1)
            hh16(g1, whh1, h1)  # depends h1(t-1): pre-runs during l0 cell
            for op in ops0:
                op()
            l1_mms_ih(g1, h0w)
            for op in cell_ops(g1, "l1", h1, c1, nc.gpsimd):
                op()
            pred_col(col)

        nar = NP - 1
        artrip = nar // AR_BODY
        rem = nar - artrip * AR_BODY
        if artrip > 0:
            with tc.For_i(0, artrip, 1, hint_engines=hints) as av:
                for u in range(AR_BODY):
                    ar_step(u, av * AR_BODY + (u + 1))
        for u in range(rem):
            ar_step(u, artrip * AR_BODY + u + 1)

        nc.vector.tensor_scalar_add(preds[:], preds_ps[:], linbcol[:])
        nc.sync.dma_start(out_ap, preds[:])


# ---------------------------------------------------------------- entry


def make_in_maps(inputs, ncores=NCORES):
    shared = prep_shared(inputs)
    return [
        dict(shared, inpT=prep_core_input(inputs["input"], c)) for c in range(ncores)
    ]


_PROG_CACHE = {}


def kernel(**inputs):
    inp = np.asarray(inputs["input"], np.float32)
    assert inp.shape == (256, 2048), inp.shape
    NP = int(inputs["num_predictions"])
    if NP not in _PROG_CACHE:
        _PROG_CACHE[NP] = build_program(T_FULL, NP)
    nc = _PROG_CACHE[NP]
    in_maps = make_in_maps(inputs)
    res = bass_utils.run_bass_kernel_spmd(nc, in_maps, core_ids=list(range(NCORES)))
    return np.concatenate([r["out"] for r in res.results], axis=0)


if __name__ == "__main__":
    import reference

    inputs = {k: np.asarray(v) for k, v in reference.setup_inputs().items()}
    out = kernel(**inputs)
    exp = np.asarray(reference.reference(**reference.setup_inputs()))
    err = np.abs(out - exp).max()
    print("absmax err", err, "rel", err / np.abs(exp).max())


# revision 10
# speedup vs baseline: 1.1179x; 1.1179x over previous
"""Trainium2 Bass kernel for CnnLSTM (conv1x1 -> 2-layer LSTM -> AR decode).

Strategy: pure data parallel over batch (B=256 -> 32 per core x 8 cores).
Device layout is "feature-major": gates live as [128 partitions = G-chunk,
32 free = batch], hidden state as [128 part = h-dim chunk, 2*32], so the
state tile directly provides matmul rhs slices and no transposes are ever
needed.  Matmul operands are fp16; all accumulation/state math is fp32.

Perf structure (v2):
- Gate biases are folded into the PE accumulation via one [8,128] x [8,256]
  indicator matmul per gate group (removes the DVE bias add + a serial hop
  from every cell).
- Warm loop is software-pipelined with a one-step skew: layer1 of step t-1
  runs concurrently with layer0 of step t (h0 double-buffered), and the two
  cells' ACT/DVE ops are emitted interleaved so the engines' in-order
  queues alternate between the two independent chains.
- AR decode feeds xt directly from h1 via the rank-1 weight matrix
  M = conv_w (x) lin_w (xt = relu(M @ h1 + (lin_b*conv_w + conv_b))),
  eliminating the prediction-row round trip (prow -> p16 -> outer product)
  from the recurrence's critical path.
"""

import numpy as np

import concourse.bacc as bacc
import concourse.bass as bass
import concourse.mybir as mybir
import concourse.tile as tile
from concourse import bass_utils
from concourse.bass import ds

F16 = mybir.dt.float16
F32 = mybir.dt.float32
AF = mybir.ActivationFunctionType
ALU = mybir.AluOpType
ET = mybir.EngineType

# PSUM gate-slot order: i,i,f,f,o,o,g,g  (PyTorch gate order along G is i,f,g,o)
GPERM = [0, 1, 2, 3, 6, 7, 4, 5]
P = 128
B = 32  # batch per core
NCORES = 8
T_FULL = 2048
WARM_BODY = 64  # warmup steps per For_i iteration
AR_BODY = 16  # AR steps per For_i iteration


# ---------------------------------------------------------------- host prep


def _relay_hh(W):
    # W [1024, 256] -> lhsT tiles [128, 2048] fp16; col block (k*8+s)*128+j
    # holds W.T[k*128+p, GPERM[s]*128+j]
    Wt = W.T.reshape(2, 128, 8, 128)
    Wt = Wt[:, :, GPERM, :]
    return np.ascontiguousarray(
        Wt.transpose(1, 0, 2, 3).reshape(128, 2048)
    ).astype(np.float16)


def _bias_lhsT(b):
    # [8, 128] f16: biasT[s, p] = b[GPERM[s]*128 + p]
    return np.ascontiguousarray(b.reshape(8, 128)[GPERM]).astype(np.float16)


def prep_shared(inputs):
    f32 = np.float32
    g = lambda n: np.asarray(inputs[n], f32)
    W_ih0, W_hh0 = g("W_ih0"), g("W_hh0")
    W_ih1, W_hh1 = g("W_ih1"), g("W_hh1")
    b0 = g("b_ih0") + g("b_hh0")
    b1 = g("b_ih1") + g("b_hh1")
    conv_w, conv_b = g("conv_w"), g("conv_b")
    lin_w, lin_b = g("lin_w"), g("lin_b")

    # x-projection weights, slot-major: wih0u[p, s*128+j] = W_ih0.T[p, GPERM[s]*128+j]
    Wt0 = W_ih0.T.reshape(64, 8, 128)[:, GPERM, :]
    wih0u = np.ascontiguousarray(Wt0.reshape(64, 1024)).astype(np.float16)

    cw2 = np.tile(conv_w, 2)
    cb2 = np.tile(conv_b, 2)

    # AR head shortcut: xt_pre = M @ h1 where M[p, j] = cw2[p] * lin_w[j].
    # lhsT chunk k is M[:, k*128:(k+1)*128].T
    Mfull = cw2[:, None] * lin_w[0][None, :]  # [128, 256]
    mwT = np.concatenate(
        [np.ascontiguousarray(Mfull[:, k * 128 : (k + 1) * 128].T) for k in range(2)],
        axis=1,
    ).astype(np.float16)

    # bias indicator rhs: bind[k, s*32+b] = 1.0 if s == k
    bind = np.zeros((8, 256), np.float16)
    for s in range(8):
        bind[s, s * 32 : (s + 1) * 32] = 1.0

    return {
        "whh0": _relay_hh(W_hh0),
        "wih1": _relay_hh(W_ih1),
        "whh1": _relay_hh(W_hh1),
        "wih0u": wih0u,
        "biasT0": _bias_lhsT(b0),
        "biasT1": _bias_lhsT(b1),
        "bind": bind,
        "mwT": mwT,
        # interleaved [cw[c], cb[c]] pairs, replicated across partitions
        "cwcb": np.ascontiguousarray(
            np.broadcast_to(
                np.stack([conv_w, conv_b], axis=1).reshape(1, 128), (128, 128)
            )
        ).astype(f32),
        "linwT": np.ascontiguousarray(lin_w[0].reshape(2, 128).T).astype(np.float16),
        "cb2col": (lin_b[0] * cw2 + cb2).astype(f32)[:, None],
        "linbcol": np.full((32, 1), lin_b[0], f32),
    }


def prep_core_input(input_full, core):
    # inpT[p, r*32+b] = input[32*core+b, 64*r + p%64], duplicated rows 64:128
    x = np.asarray(input_full, np.float32)[32 * core : 32 * core + 32]
    x = x.reshape(32, 32, 64)  # [b, r, k]
    one = x.transpose(2, 1, 0).reshape(64, 1024)  # [k, (r b)]
    return np.ascontiguousarray(np.concatenate([one, one], axis=0))


# ---------------------------------------------------------------- device IR


def build_program(T=T_FULL, NP=512, mode="full"):
    assert T % WARM_BODY == 0 and T <= T_FULL
    assert 2 <= NP <= 512
    nc = bacc.Bacc("TRN2", debug=False, enable_asserts=False, num_devices=NCORES)

    def din(name, shape, dt):
        return nc.dram_tensor(name, list(shape), dt, kind="ExternalInput").ap()

    t = {
        "whh0": din("whh0", (128, 2048), F16),
        "wih1": din("wih1", (128, 2048), F16),
        "whh1": din("whh1", (128, 2048), F16),
        "wih0u": din("wih0u", (64, 1024), F16),
        "inpT": din("inpT", (128, 1024), F32),
        "biasT0": din("biasT0", (8, 128), F16),
        "biasT1": din("biasT1", (8, 128), F16),
        "bind": din("bind", (8, 256), F16),
        "mwT": din("mwT", (128, 256), F16),
        "cwcb": din("cwcb", (128, 128), F32),
        "linwT": din("linwT", (128, 2), F16),
        "cb2col": din("cb2col", (128, 1), F32),
        "linbcol": din("linbcol", (32, 1), F32),
    }
    if mode == "warm":
        out_ap = nc.dram_tensor("out", [128, 128], F32, kind="ExternalOutput").ap()
    else:
        out_ap = nc.dram_tensor("out", [32, NP], F32, kind="ExternalOutput").ap()

    with tile.TileContext(nc) as tc:
        _emit(tc, nc, t, out_ap, T, NP, mode)
    nc.compile()
    return nc


def _emit(tc, nc, t, out_ap, T, NP, mode="full"):
    import contextlib
    from itertools import zip_longest

    with contextlib.ExitStack() as ctx:
        const = ctx.enter_context(tc.tile_pool(name="const", bufs=1))

        def load(name, shape, dt):
            tl = const.tile(list(shape), dt, tag=name)
            nc.sync.dma_start(tl[:], t[name])
            return tl

        whh0 = load("whh0", (128, 2048), F16)
        wih1 = load("wih1", (128, 2048), F16)
        whh1 = load("whh1", (128, 2048), F16)
        wih0u = load("wih0u", (64, 1024), F16)
        inpT = load("inpT", (128, 1024), F32)
        biasT0 = load("biasT0", (8, 128), F16)
        biasT1 = load("biasT1", (8, 128), F16)
        bind = load("bind", (8, 256), F16)
        mwT = load("mwT", (128, 256), F16)
        cwcb = load("cwcb", (128, 128), F32)
        linwT = load("linwT", (128, 2), F16)
        cb2col = load("cb2col", (128, 1), F32)
        linbcol = load("linbcol", (32, 1), F32)

        # persistent state; h0 double-buffered for the warm-loop skew
        h0b = [
            const.tile([128, 64], F16, tag=f"h0_{i}", name=f"h0_{i}")
            for i in range(2)
        ]
        c0 = const.tile([128, 64], F32, tag="c0")
        h1 = const.tile([128, 64], F16, tag="h1")
        c1 = const.tile([128, 64], F32, tag="c1")
        for st in (h0b[0], h0b[1], c0, h1, c1):
            nc.vector.memset(st[:], 0.0)
        preds = const.tile([32, NP], F32, tag="preds")

        gpool = ctx.enter_context(tc.tile_pool(name="gates", bufs=2, space="PSUM"))
        spool = ctx.enter_context(tc.tile_pool(name="sg", bufs=2))
        tpool = ctx.enter_context(tc.tile_pool(name="tmp", bufs=2))
        xpool = ctx.enter_context(tc.tile_pool(name="xt", bufs=2))
        appool = ctx.enter_context(tc.tile_pool(name="arp", bufs=1, space="PSUM"))

        def cell_ops(g, tagp, h_dst, c_st, v):
            """Return the cell's ACT/elementwise ops as closures so two cells
            can be emitted interleaved (each engine's in-order queue then
            alternates between the two independent chains).  `v` picks the
            elementwise engine (nc.vector for layer0, nc.gpsimd for layer1) so
            the two chains don't share a single elementwise queue."""
            sg = spool.tile([128, 192], F32, tag=tagp + "s")
            gt = tpool.tile([128, 64], F32, tag=tagp + "g")
            m1 = tpool.tile([128, 64], F32, tag=tagp + "m1")
            m2 = tpool.tile([128, 64], F32, tag=tagp + "m2")
            tcc = tpool.tile([128, 64], F32, tag=tagp + "t")
            return [
                lambda: nc.scalar.activation(sg[:], g[:, 0:192], AF.Sigmoid),
                lambda: nc.scalar.activation(gt[:], g[:, 192:256], AF.Tanh),
                lambda: v.tensor_mul(m1[:], sg[:, 64:128], c_st[:]),
                lambda: v.tensor_mul(m2[:], sg[:, 0:64], gt[:]),
                lambda: v.tensor_add(c_st[:], m1[:], m2[:]),
                lambda: nc.scalar.activation(tcc[:], c_st[:], AF.Tanh),
                lambda: v.tensor_mul(h_dst[:], sg[:, 128:192], tcc[:]),
            ]

        def hh16(g, w, rhs_tile):
            for s in range(8):
                for k in range(2):
                    nc.tensor.matmul(
                        g[:, s * 32 : (s + 1) * 32],
                        lhsT=w[:, (k * 8 + s) * 128 : (k * 8 + s + 1) * 128],
                        rhs=rhs_tile[:, k * 32 : (k + 1) * 32],
                        start=False,
                        stop=False,
                        skip_group_check=True,
                    )

        def bias_mm(g, biasT):
            nc.tensor.matmul(
                g[:], lhsT=biasT[:], rhs=bind[:], start=True, stop=False,
                skip_group_check=True,
            )

        def l0_mms_x(g, xt):
            for s in range(8):
                nc.tensor.matmul(
                    g[:, s * 32 : (s + 1) * 32],
                    lhsT=wih0u[:, s * 128 : (s + 1) * 128],
                    rhs=xt[0:64, :],
                    start=False,
                    stop=(s == 7),
                    skip_group_check=True,
                )

        def l1_mms_ih(g, h0r, last_stop=True):
            for s in range(8):
                for k in range(2):
                    nc.tensor.matmul(
                        g[:, s * 32 : (s + 1) * 32],
                        lhsT=wih1[:, (k * 8 + s) * 128 : (k * 8 + s + 1) * 128],
                        rhs=h0r[:, k * 32 : (k + 1) * 32],
                        start=False,
                        stop=(last_stop and s == 7 and k == 1),
                        skip_group_check=True,
                    )

        def emit_l0(u, xcur, h0r, h0w):
            g = gpool.tile([128, 256], F32, tag="g0")
            bias_mm(g, biasT0)
            hh16(g, whh0, h0r)
            for s in range(8):
                nc.tensor.matmul(
                    g[:, s * 32 : (s + 1) * 32],
                    lhsT=wih0u[:, s * 128 : (s + 1) * 128],
                    rhs=xcur[0:64, u * 32 : (u + 1) * 32],
                    start=False,
                    stop=(s == 7),
                    skip_group_check=True,
                )
            return cell_ops(g, "l0", h0w, c0, nc.vector)

        def emit_l1(h0r):
            g = gpool.tile([128, 256], F32, tag="g1")
            bias_mm(g, biasT1)
            hh16(g, whh1, h1)
            l1_mms_ih(g, h0r)
            return cell_ops(g, "l1", h1, c1, nc.vector)

        preds_ps = appool.tile([32, NP], F32, tag="predps")

        def pred_col(col):
            for k in range(2):
                nc.tensor.matmul(
                    preds_ps[:, ds(col, 1)],
                    lhsT=h1[:, k * 32 : (k + 1) * 32],
                    rhs=linwT[:, k : k + 1],
                    start=(k == 0),
                    stop=(k == 1),
                    skip_group_check=True,
                )

        hints = (ET.PE, ET.DVE, ET.Activation)

        wtrip = T // WARM_BODY
        with tc.For_i(0, wtrip, 1, hint_engines=hints) as iv:
            # ACT scale/bias operands do not support register offsets on HW,
            # so snapshot this body's two (cw, cb) pairs into static tiles
            # via DMA (register offsets on DMA are fine).
            curs = []
            for half in range(2):
                cur = xpool.tile([128, 2], F32, tag="cwcur")
                nc.sync.dma_start(cur[:], cwcb[:, ds(iv * 4 + half * 2, 2)])
                curs.append(cur)
            # conv+relu for the whole body in two bulk ACT ops (instead of a
            # per-step [128,32] ReLU on the cell chains' shared ACT queue)
            xcur = xpool.tile([128, 2048], F16, tag="xcur")
            for half in range(2):
                nc.scalar.activation(
                    xcur[:, half * 1024 : (half + 1) * 1024],
                    inpT[:],
                    AF.Relu,
                    bias=curs[half][:, 1:2],
                    scale=curs[half][:, 0:1],
                )
            for u in range(WARM_BODY):
                ops0 = emit_l0(u, xcur, h0b[(u - 1) % 2], h0b[u % 2])
                # layer1 for step u-1 (reads h0 written by step u-1)
                ops1 = emit_l1(h0b[(u - 1) % 2]) if u > 0 else None
                for a, b in zip_longest(ops0, ops1 or []):
                    if a is not None:
                        a()
                    if b is not None:
                        b()
            # tail: layer1 of the body's last step
            for op in emit_l1(h0b[(WARM_BODY - 1) % 2]):
                op()

        if mode == "warm":
            dbg = const.tile([128, 128], F32, tag="dbg")
            nc.vector.tensor_copy(dbg[:, 0:64], h1[:])
            nc.vector.tensor_copy(dbg[:, 64:128], c1[:])
            nc.sync.dma_start(out_ap, dbg[:])
            return

        pred_col(0)

        # ----- AR decode: serial chain, but with the M-shortcut head and
        # matmul groups ordered so PE pre-runs whatever is data-ready.
        def ar_step(j, col):
            h0r, h0w = h0b[(j + 1) % 2], h0b[j % 2]
            g0 = gpool.tile([128, 256], F32, tag="g0")
            bias_mm(g0, biasT0)
            hh16(g0, whh0, h0r)  # depends h0(t-1): pre-runs during prev cell
            # xt from h1(t-1) via rank-1 M
            xps = appool.tile([128, 32], F32, tag="xps")
            for k in range(2):
                nc.tensor.matmul(
                    xps[:],
                    lhsT=mwT[:, k * 128 : (k + 1) * 128],
                    rhs=h1[:, k * 32 : (k + 1) * 32],
                    start=(k == 0),
                    stop=(k == 1),
                    skip_group_check=True,
                )
            xt = xpool.tile([128, 32], F16, tag="xt")
            nc.scalar.activation(xt[:], xps[:], AF.Relu, bias=cb2col[:])
            l0_mms_x(g0, xt)
            ops0 = cell_ops(g0, "l0", h0w, c0, nc.vector)
            g1 = gpool.tile([128, 256], F32, tag="g1")
            bias_mm(g1, biasT1)
            hh16(g1, whh1, h1)  # depends h1(t-1): pre-runs during l0 cell
            for op in ops0:
                op()
            l1_mms_ih(g1, h0w)
            for op in cell_ops(g1, "l1", h1, c1, nc.vector):
                op()
            pred_col(col)

        nar = NP - 1
        artrip = nar // AR_BODY
        rem = nar - artrip * AR_BODY
        if artrip > 0:
            with tc.For_i(0, artrip, 1, hint_engines=hints) as av:
                for u in range(AR_BODY):
                    ar_step(u, av * AR_BODY + (u + 1))
        for u in range(rem):
            ar_step(u, artrip * AR_BODY + u + 1)

        nc.vector.tensor_scalar_add(preds[:], preds_ps[:], linbcol[:])
        nc.sync.dma_start(out_ap, preds[:])


# ---------------------------------------------------------------- entry


def make_in_maps(inputs, ncores=NCORES):
    shared = prep_shared(inputs)
    return [
        dict(shared, inpT=prep_core_input(inputs["input"], c)) for c in range(ncores)
    ]


_PROG_CACHE = {}


def kernel(**inputs):
    inp = np.asarray(inputs["input"], np.float32)
    assert inp.shape == (256, 2048), inp.shape
    NP = int(inputs["num_predictions"])
    if NP not in _PROG_CACHE:
        _PROG_CACHE[NP] = build_program(T_FULL, NP)
    nc = _PROG_CACHE[NP]
    in_maps = make_in_maps(inputs)
    res = bass_utils.run_bass_kernel_spmd(nc, in_maps, core_ids=list(range(NCORES)))
    return np.concatenate([r["out"] for r in res.results], axis=0)


if __name__ == "__main__":
    import reference

    inputs = {k: np.asarray(v) for k, v in reference.setup_inputs().items()}
    out = kernel(**inputs)
    exp = np.asarray(reference.reference(**reference.setup_inputs()))
    err = np.abs(out - exp).max()
    print("absmax err", err, "rel", err / np.abs(exp).max())
